# revision 1
# baseline (speedup 1.0000x reference)
"""Trainium2 Bass kernel for nn_BayesianLayer (Bayesian linear layer).

Math (per batch row b):
    sigma      = softplus(ro)                          # (IN, OUT)
    weights_b  = eps_b * sigma + mu                    # (IN, OUT)
    bias_b     = eps_bias_b * softplus(ro_bias) + mu_bias
    out_b      = x_b @ weights_b + bias_b              # (OUT,)

Sharding: data-parallel over the batch dim across 8 NeuronCores
(16 rows each); mu/ro/biases replicated.

Per-core device kernel (DMA-bound; ~72.8 MB HBM traffic => ~210-225 us):
  - sigma = softplus(ro) = Ln(Exp(ro)+1) computed on ScalarE.
  - the eps term streams in [128, 1024] tiles (i on partitions, one
    k-block per tile). VectorE computes (eps * x[b,i]) * sigma in one
    scalar_tensor_tensor op (x enters as an exact per-partition f32
    scalar), writing float32r. TensorE reduces over i with an exact
    all-ones [128,1] stationary at full PE rate (f32r = 1 cycle/row),
    accumulating the 8 k-blocks of each batch row into a [1, 1024] PSUM
    group (4 live groups = 8 banks, pool-rotated).
  - the mu term (x @ mu) is one M=16 full-precision fp32 matmul phase,
    merged into the bias rows, so it adds no per-sample PE work.
  - per batch row: VectorE adds PSUM + bias row and the output row is
    DMA'd out immediately.
  - eps DMAs alternate the two HWDGE rings (sync/scalar); all preamble
    and small transfers use SWDGE (gpsimd) to avoid head-of-line
    blocking of the streaming rings.
"""

import numpy as np
from contextlib import ExitStack

import concourse.mybir as mybir
import concourse.tile as tile
from concourse import bacc
from concourse.bass_utils import run_bass_kernel_spmd

B, IN, OUT = 128, 1024, 1024
N_CORES = 8
BP = B // N_CORES          # 16 batch rows per core
P = 128                    # partitions
KB = IN // P               # 8 k-blocks
NHALF = 512                # fp32-family matmul max moving free dim
CHUNK_K = 1                # k-blocks per eps chunk (one x scalar per STT)
N_CHUNKS = KB // CHUNK_K

f32 = mybir.dt.float32
f32r = mybir.dt.float32r
MULT = mybir.AluOpType.mult
ADD = mybir.AluOpType.add
ACT = mybir.ActivationFunctionType

EPS_BUFS = 6               # rounded-tile slots (raw eps slots = EPS_BUFS + 2)
BLK = 2                    # batch rows per pipelined block (4 PSUM groups max)
REP = 1                    # body repetitions (>1 only for timing experiments)

_compiled = {}


def _softplus_tiles(nc, out_sl, in_sl):
    """out_sl = softplus(in_sl) = ln(1 + exp(x)).

    Direct form: safe for |x| <~ 80 (inputs here are N(0,1)).
    """
    nc.scalar.activation(out_sl, in_sl, ACT.Exp)
    nc.scalar.activation(out_sl, out_sl, ACT.Ln, bias=1.0)


def build(rep=None):
    rep = REP if rep is None else rep
    nc = bacc.Bacc("TRN2", debug=False, enable_asserts=False)

    eps_d = nc.dram_tensor("eps", (BP, IN, OUT), f32, kind="ExternalInput").ap()
    xT_d = nc.dram_tensor("xT", (IN, BP), f32, kind="ExternalInput").ap()
    mu_d = nc.dram_tensor("mu", (IN, OUT), f32, kind="ExternalInput").ap()
    ro_d = nc.dram_tensor("ro", (IN, OUT), f32, kind="ExternalInput").ap()
    eb_d = nc.dram_tensor("ebias", (BP, OUT), f32, kind="ExternalInput").ap()
    rb_d = nc.dram_tensor("robias", (BP, OUT), f32, kind="ExternalInput").ap()
    mb_d = nc.dram_tensor("mubias", (BP, OUT), f32, kind="ExternalInput").ap()
    out_d = nc.dram_tensor("out", (BP, OUT), f32, kind="ExternalOutput").ap()

    # [p, k*OUT + o] layouts (i = k*128 + p on partitions)
    ro_r = ro_d.rearrange("(k p) o -> p k o", p=P)
    mu_r = mu_d.rearrange("(k p) o -> p k o", p=P)
    eps_r = eps_d.rearrange("b (k p) o -> b p k o", p=P)
    xT_r = xT_d.rearrange("(k p) m -> p k m", p=P)

    with tile.TileContext(nc) as tc, ExitStack() as ctx:
        consts = ctx.enter_context(tc.tile_pool(name="consts", bufs=1))
        small = ctx.enter_context(tc.tile_pool(name="small", bufs=1))
        eps_pool = ctx.enter_context(tc.tile_pool(name="eps_pool", bufs=EPS_BUFS))
        psum_pool = ctx.enter_context(tc.tile_pool(name="psum", bufs=1, space="PSUM"))

        for _rep in range(rep):
            # ---- constants / preamble ----
            # x columns first (tiny; needed by every matmul)
            xT_f32 = consts.tile([P, KB, BP], f32)
            nc.gpsimd.dma_start(xT_f32[:], xT_r)
            # exact all-ones stationary for the eps-term GEMVs (x is folded
            # into the VectorE product as a per-partition scalar instead)
            ones_f32 = small.tile([P, 1], f32)
            nc.vector.memset(ones_f32[:], 1.0)
            ones_r = consts.tile([P, 1], f32r)
            nc.vector.tensor_copy(ones_r[:], ones_f32[:])

            # sigma (softplus on ACT) and mu (plain f32), interleaved per
            # k-block so chunk 0 is ready as early as possible. The mu term
            # (x @ mu for all 16 rows) is one M=16 full-fp32 matmul phase,
            # accumulated chunk by chunk into a shared psum slot.
            sigma_all = consts.tile([P, KB, OUT], f32)
            mu_all = consts.tile([P, KB, OUT], f32)
            psum_mu = psum_pool.tile([BP, OUT], f32, tag="pb", bufs=4, name="psum_mu")
            for c in range(KB):
                ro_t = small.tile([P, OUT], f32, tag="pre_tmp", bufs=6, name="ro_t")
                nc.gpsimd.dma_start(ro_t[:], ro_r[:, c, :])
                _softplus_tiles(nc, sigma_all[:, c, :], ro_t[:])
                nc.gpsimd.dma_start(mu_all[:, c, :], mu_r[:, c, :])
                for h in range(2):
                    nc.tensor.matmul(
                        psum_mu[:, h * NHALF : (h + 1) * NHALF],
                        xT_f32[:, c, :],
                        mu_all[:, c, h * NHALF : (h + 1) * NHALF],
                        start=(c == 0),
                        stop=(c == KB - 1),
                    )
            mu_out16 = small.tile([BP, OUT], f32)
            nc.vector.tensor_copy(mu_out16[:], psum_mu[:])

            # ---- bias rows: bias16 = ebias * softplus(robias) + mubias ----
            eb16 = small.tile([BP, OUT], f32)
            nc.gpsimd.dma_start(eb16[:], eb_d)
            rb16 = small.tile([BP, OUT], f32)
            nc.gpsimd.dma_start(rb16[:], rb_d)
            mb16 = small.tile([BP, OUT], f32)
            nc.gpsimd.dma_start(mb16[:], mb_d)
            sb16 = small.tile([BP, OUT], f32)
            _softplus_tiles(nc, sb16[:], rb16[:])
            nc.vector.tensor_tensor(eb16[:], eb16[:], sb16[:], MULT)
            nc.vector.tensor_tensor(eb16[:], eb16[:], mb16[:], ADD)
            nc.vector.tensor_tensor(eb16[:], eb16[:], mu_out16[:], ADD)

            # ---- main loop: blocks of BLK batch rows, chunk-major inside
            # (the block's first eps multiplies only need sigma chunk 0, so
            # sigma production stays ahead; <=4 live PSUM groups = 8 banks).
            # The last two rows run as single-row blocks to shorten the
            # end-of-kernel critical chain.
            blocks = [
                list(range(s, min(s + BLK, BP))) for s in range(0, BP - 2, BLK)
            ] + [[BP - 2], [BP - 1]]
            for blk in blocks:
                prows = {
                    b: psum_pool.tile([1, OUT], f32, tag="pb", bufs=4, name="prow")
                    for b in blk
                }
                for c in range(N_CHUNKS):
                    ksl = slice(c * CHUNK_K, (c + 1) * CHUNK_K)
                    for b in blk:
                        et = eps_pool.tile(
                            [P, CHUNK_K, OUT], f32, tag="eps_t", name="et",
                            bufs=EPS_BUFS + 2,
                        )
                        dma_eng = nc.sync if (b + c) % 2 == 0 else nc.scalar
                        dma_eng.dma_start(et[:], eps_r[b][:, ksl, :])
                        er = eps_pool.tile(
                            [P, CHUNK_K, OUT], f32r, tag="eps_r", name="er"
                        )
                        # er = (eps * x[b, i]) * sigma, one f32r rounding
                        nc.vector.scalar_tensor_tensor(
                            er[:],
                            et[:],
                            xT_f32[:, c, b : b + 1],
                            sigma_all[:, ksl, :],
                            MULT,
                            MULT,
                        )
                        for ks in range(CHUNK_K):
                            k = c * CHUNK_K + ks
                            for h in range(2):
                                pr = prows[b][:, h * NHALF : (h + 1) * NHALF]
                                nc.tensor.matmul(
                                    pr,
                                    ones_r[:],
                                    er[:, ks, h * NHALF : (h + 1) * NHALF],
                                    start=(k == 0),
                                    stop=(k == KB - 1),
                                )
                # tails: out row = psum + bias (partition 0), stream out
                for b in blk:
                    bias_b = eps_pool.tile(
                        [1, OUT], f32, tag="bias_b", bufs=3, name="bias_b"
                    )
                    nc.gpsimd.dma_start(bias_b[:], eb16[b : b + 1, :])
                    out_b = eps_pool.tile(
                        [1, OUT], f32, tag="out_b", bufs=3, name="out_b"
                    )
                    nc.vector.tensor_tensor(out_b[:], prows[b][:], bias_b[:], ADD)
                    nc.sync.dma_start(out_d[b : b + 1, :], out_b[:])

    nc.compile()
    return nc


def get_nc(rep=None):
    rep = REP if rep is None else rep
    key = (CHUNK_K, EPS_BUFS, BLK, rep)
    if key not in _compiled:
        _compiled[key] = build(rep)
    return _compiled[key]


def make_in_maps(x, eps, eps_bias, mu, ro, mu_bias, ro_bias):
    x = np.ascontiguousarray(np.asarray(x, dtype=np.float32))
    eps = np.asarray(eps, dtype=np.float32)
    eps_bias = np.asarray(eps_bias, dtype=np.float32)
    mu = np.ascontiguousarray(np.asarray(mu, dtype=np.float32))
    ro = np.ascontiguousarray(np.asarray(ro, dtype=np.float32))
    mu_b = np.ascontiguousarray(
        np.broadcast_to(np.asarray(mu_bias, dtype=np.float32).reshape(1, OUT), (BP, OUT))
    )
    ro_b = np.ascontiguousarray(
        np.broadcast_to(np.asarray(ro_bias, dtype=np.float32).reshape(1, OUT), (BP, OUT))
    )
    in_maps = []
    for c in range(N_CORES):
        sl = slice(c * BP, (c + 1) * BP)
        in_maps.append(
            {
                "eps": np.ascontiguousarray(eps[sl]),
                "xT": np.ascontiguousarray(x[sl].T),
                "mu": mu,
                "ro": ro,
                "ebias": np.ascontiguousarray(eps_bias[sl]),
                "robias": ro_b,
                "mubias": mu_b,
            }
        )
    return in_maps


def run(trace=False, **inputs):
    nc = get_nc()
    in_maps = make_in_maps(**inputs)
    res = run_bass_kernel_spmd(
        nc, in_maps, core_ids=list(range(N_CORES)), trace=trace
    )
    out = np.concatenate([r["out"] for r in res.results], axis=0)
    return out, res


def kernel(**inputs) -> np.ndarray:
    out, _ = run(trace=False, **inputs)
    return out



# revision 3
# speedup vs baseline: 1.4150x; 1.4150x over previous
"""Trainium2 Bass kernel for nn_BayesianLayer (Bayesian linear layer).

Math (per batch row b):
    sigma      = softplus(ro)                          # (IN, OUT)
    weights_b  = eps_b * sigma + mu                    # (IN, OUT)
    bias_b     = eps_bias_b * softplus(ro_bias) + mu_bias
    out_b      = x_b @ weights_b + bias_b              # (OUT,)

Sharding: data-parallel over the batch dim across 8 NeuronCores
(16 rows each); mu/ro/biases replicated.

The kernel is DMA-bound on streaming eps, so eps/mu/ro/x/biases are
staged host-side in fp16 (the rel-err budget is 2e-2; fp16 staging
costs ~3e-4). Per-core HBM traffic drops from ~72.8 MB to ~38 MB.

Per-core device kernel:
  - sigma = softplus(ro) = Ln(Exp(ro)+1) on ScalarE (f32 intermediate,
    fp16 result kept in SBUF for all 8 k-blocks).
  - eps streams in [128, 4*1024] fp16 tiles (i on partitions, 4
    k-blocks per tile). VectorE computes eps * sigma with a fp16
    tensor_tensor (2x DVE fast mode). TensorE contracts each k-block
    with the x column as the (free) ldweights stationary, accumulating
    into a [1, 1024] PSUM row per batch sample.
  - the mu term (x @ mu) is one M=16 fp16 matmul phase; its PSUM
    result is folded into the bias rows (bias = eps_bias *
    softplus(ro_bias) + mu_bias + x@mu), which are added into each
    sample's PSUM row by a 16-partition matmul against an identity
    column, so the PSUM row is complete when the last matmul retires.
  - output rows are DMA'd straight out of PSUM.
  - preamble/small transfers ride the SWDGE (gpsimd) queue; eps and
    output use the HWDGE sync ring.
"""

import numpy as np
from contextlib import ExitStack

import concourse.mybir as mybir
import concourse.tile as tile
from concourse import bacc
from concourse.bass_utils import run_bass_kernel_spmd

B, IN, OUT = 128, 1024, 1024
N_CORES = 8
BP = B // N_CORES          # 16 batch rows per core
P = 128                    # partitions
KB = IN // P               # 8 k-blocks
NHALF = 512                # PSUM-bank-sized matmul moving free dim
CHUNK_K = 4                # k-blocks per eps chunk
N_CHUNKS = KB // CHUNK_K

f32 = mybir.dt.float32
f16 = mybir.dt.float16
MULT = mybir.AluOpType.mult
ADD = mybir.AluOpType.add
ACT = mybir.ActivationFunctionType

EPS_BUFS = 5               # eps stream tile slots
ER_BUFS = 3                # eps*sigma product slots
REP = 1                    # body repetitions (>1 only for timing experiments)

_compiled = {}


def build(rep=None):
    rep = REP if rep is None else rep
    nc = bacc.Bacc("TRN2", debug=False, enable_asserts=False)

    eps_d = nc.dram_tensor("eps", (BP, IN, OUT), f16, kind="ExternalInput").ap()
    xTp_d = nc.dram_tensor("xTp", (P, KB, BP), f16, kind="ExternalInput").ap()
    mu_d = nc.dram_tensor("mu", (KB, P, OUT), f16, kind="ExternalInput").ap()
    ro_d = nc.dram_tensor("ro", (KB, P, OUT), f16, kind="ExternalInput").ap()
    eb_d = nc.dram_tensor("ebias", (BP, OUT), f16, kind="ExternalInput").ap()
    rb_d = nc.dram_tensor("robias", (BP, OUT), f16, kind="ExternalInput").ap()
    mb_d = nc.dram_tensor("mubias", (BP, OUT), f16, kind="ExternalInput").ap()
    id_d = nc.dram_tensor("ident", (BP, BP), f16, kind="ExternalInput").ap()
    out_d = nc.dram_tensor("out", (BP, OUT), f32, kind="ExternalOutput").ap()

    # eps as [b][p, k, o] (i = k*128 + p on partitions)
    eps_r = eps_d.rearrange("b (k p) o -> b p k o", p=P)

    with tile.TileContext(nc) as tc, ExitStack() as ctx:
        consts = ctx.enter_context(tc.tile_pool(name="consts", bufs=1))
        small = ctx.enter_context(tc.tile_pool(name="small", bufs=1))
        eps_pool = ctx.enter_context(tc.tile_pool(name="eps_pool", bufs=1))
        psum_pool = ctx.enter_context(tc.tile_pool(name="psum", bufs=1, space="PSUM"))

        for _rep in range(rep):
            # ---- constants / preamble ----
            xTp = consts.tile([P, KB, BP], f16)
            nc.gpsimd.dma_start(xTp[:], xTp_d)
            ident = consts.tile([BP, BP], f16)
            nc.gpsimd.dma_start(ident[:], id_d)

            # sigma = softplus(ro), fp16, kept resident for all k-blocks
            sigma_all = consts.tile([P, KB, OUT], f16)
            for k in range(KB):
                ro_t = small.tile([P, OUT], f16, tag="ro_t", bufs=3, name="ro_t")
                nc.gpsimd.dma_start(ro_t[:], ro_d[k])
                exp_t = small.tile([P, OUT], f32, tag="exp_t", bufs=2, name="exp_t")
                nc.scalar.activation(exp_t[:], ro_t[:], ACT.Exp)
                nc.scalar.activation(sigma_all[:, k, :], exp_t[:], ACT.Ln, bias=1.0)

            # mu term: psum_mu[m, o] = sum_i x[m, i] * mu[i, o], M=16 fp16 phase
            psum_mu = psum_pool.tile([BP, OUT], f32, tag="pb", bufs=4, name="psum_mu")
            for k in range(KB):
                mu_t = small.tile([P, OUT], f16, tag="mu_t", bufs=3, name="mu_t")
                nc.gpsimd.dma_start(mu_t[:], mu_d[k])
                for h in range(2):
                    nc.tensor.matmul(
                        psum_mu[:, h * NHALF : (h + 1) * NHALF],
                        xTp[:, k, :],
                        mu_t[:, h * NHALF : (h + 1) * NHALF],
                        start=(k == 0),
                        stop=(k == KB - 1),
                    )
            mu_s = small.tile([BP, OUT], f16)
            nc.scalar.activation(mu_s[:], psum_mu[:], ACT.Copy)

            # bias rows: base16 = ebias * softplus(robias) + mubias + x@mu
            eb16 = small.tile([BP, OUT], f16)
            nc.gpsimd.dma_start(eb16[:], eb_d)
            rb16 = small.tile([BP, OUT], f16)
            nc.gpsimd.dma_start(rb16[:], rb_d)
            mb16 = small.tile([BP, OUT], f16)
            nc.gpsimd.dma_start(mb16[:], mb_d)
            ebx = small.tile([BP, OUT], f32)
            nc.scalar.activation(ebx[:], rb16[:], ACT.Exp)
            sb16 = small.tile([BP, OUT], f16)
            nc.scalar.activation(sb16[:], ebx[:], ACT.Ln, bias=1.0)
            base16 = small.tile([BP, OUT], f16)
            nc.vector.tensor_tensor(base16[:], eb16[:], sb16[:], MULT)
            nc.vector.tensor_tensor(base16[:], base16[:], mb16[:], ADD)
            nc.vector.tensor_tensor(base16[:], base16[:], mu_s[:], ADD)

            # ---- main loop: one PSUM row per batch sample, eps streamed in
            # CHUNK_K k-block tiles; eps*sigma on VectorE (fp16 2x mode),
            # contraction + bias add on TensorE.
            for b in range(BP):
                prow = psum_pool.tile([1, OUT], f32, tag="pb", bufs=4, name="prow")
                for c in range(N_CHUNKS):
                    ksl = slice(c * CHUNK_K, (c + 1) * CHUNK_K)
                    et = eps_pool.tile(
                        [P, CHUNK_K, OUT], f16, tag="eps_t", name="et", bufs=EPS_BUFS
                    )
                    nc.sync.dma_start(et[:], eps_r[b][:, ksl, :])
                    er = eps_pool.tile(
                        [P, CHUNK_K, OUT], f16, tag="eps_r", name="er", bufs=ER_BUFS
                    )
                    nc.vector.tensor_tensor(er[:], et[:], sigma_all[:, ksl, :], MULT)
                    for kk in range(CHUNK_K):
                        k = c * CHUNK_K + kk
                        for h in range(2):
                            nc.tensor.matmul(
                                prow[:, h * NHALF : (h + 1) * NHALF],
                                xTp[:, k, b : b + 1],
                                er[:, kk, h * NHALF : (h + 1) * NHALF],
                                start=(k == 0),
                                stop=False,
                            )
                # fold bias row b into the PSUM row: ident[:, b] selects it
                for h in range(2):
                    nc.tensor.matmul(
                        prow[:, h * NHALF : (h + 1) * NHALF],
                        ident[:, b : b + 1],
                        base16[:, h * NHALF : (h + 1) * NHALF],
                        start=False,
                        stop=True,
                    )
                orow = eps_pool.tile([1, OUT], f32, tag="orow", bufs=3, name="orow")
                nc.scalar.activation(orow[:], prow[:], ACT.Copy)
                nc.sync.dma_start(out_d[b : b + 1, :], orow[:])

    nc.compile()
    return nc


def get_nc(rep=None):
    rep = REP if rep is None else rep
    key = (CHUNK_K, EPS_BUFS, ER_BUFS, rep)
    if key not in _compiled:
        _compiled[key] = build(rep)
    return _compiled[key]


def make_in_maps(x, eps, eps_bias, mu, ro, mu_bias, ro_bias):
    x = np.asarray(x, dtype=np.float32)
    eps = np.asarray(eps)
    eps_bias = np.asarray(eps_bias, dtype=np.float32)
    mu16 = np.ascontiguousarray(
        np.asarray(mu, dtype=np.float16).reshape(KB, P, OUT)
    )
    ro16 = np.ascontiguousarray(
        np.asarray(ro, dtype=np.float16).reshape(KB, P, OUT)
    )
    mu_b = np.ascontiguousarray(
        np.broadcast_to(
            np.asarray(mu_bias, dtype=np.float16).reshape(1, OUT), (BP, OUT)
        )
    )
    ro_b = np.ascontiguousarray(
        np.broadcast_to(
            np.asarray(ro_bias, dtype=np.float16).reshape(1, OUT), (BP, OUT)
        )
    )
    ident = np.eye(BP, dtype=np.float16)
    in_maps = []
    for c in range(N_CORES):
        sl = slice(c * BP, (c + 1) * BP)
        # x rows for this core as [p, k, m]: x[sl].T is (IN, BP) = (k*P, m)
        xTp = np.ascontiguousarray(
            x[sl].T.astype(np.float16).reshape(KB, P, BP).transpose(1, 0, 2)
        )
        in_maps.append(
            {
                "eps": np.ascontiguousarray(eps[sl], dtype=np.float16),
                "xTp": xTp,
                "mu": mu16,
                "ro": ro16,
                "ebias": np.ascontiguousarray(eps_bias[sl], dtype=np.float16),
                "robias": ro_b,
                "mubias": mu_b,
                "ident": ident,
            }
        )
    return in_maps


def run(trace=False, **inputs):
    nc = get_nc()
    in_maps = make_in_maps(**inputs)
    res = run_bass_kernel_spmd(
        nc, in_maps, core_ids=list(range(N_CORES)), trace=trace
    )
    out = np.concatenate([r["out"] for r in res.results], axis=0)
    return out, res


def kernel(**inputs) -> np.ndarray:
    out, _ = run(trace=False, **inputs)
    return out


# revision 5
# speedup vs baseline: 1.6526x; 1.1679x over previous
"""Trainium2 Bass kernel for nn_BayesianLayer (Bayesian linear layer).

Math (per batch row b):
    sigma      = softplus(ro)                          # (IN, OUT)
    weights_b  = eps_b * sigma + mu                    # (IN, OUT)
    bias_b     = eps_bias_b * softplus(ro_bias) + mu_bias
    out_b      = x_b @ weights_b + bias_b              # (OUT,)

Sharding: data-parallel over the batch dim across 8 NeuronCores
(16 rows each); mu/ro/biases replicated.

The kernel is DMA-bound on streaming eps, so eps/mu/ro/x/biases are
staged host-side in fp16 (the rel-err budget is 2e-2; fp16 staging
costs ~5e-4). Per-core HBM traffic drops from ~72.8 MB to ~38 MB.

Per-core device kernel:
  - ro and mu arrive as one 2 MB HWDGE DMA each, ahead of the eps
    stream on the sync ring; sigma = native Softplus on ScalarE, one
    [128, 1024] fp16 pass per k-block (single act table, no reloads).
  - eps streams in [128, 4*1024] fp16 tiles (i on partitions, 4
    k-blocks per tile, 8 rotating slots so the slot-free semaphores
    run well ahead of the DMA ring). VectorE computes eps * sigma
    with a fp16 tensor_tensor (2x DVE fast mode). TensorE contracts
    each k-block with the sample's x column as the (free-to-load)
    stationary, accumulating into a [1, 1024] PSUM row.
  - the mu term (x @ mu) is one M=16 fp16 matmul phase; its PSUM
    result is folded into the bias rows (bias = eps_bias *
    softplus(ro_bias) + mu_bias + x@mu), which are added into each
    sample's PSUM row by a 16-partition matmul against an identity
    column, so the PSUM row is complete when its last matmul retires.
  - PSUM rows leave via a ScalarE copy + DMA on the scalar ring
    (keeping the sync ring free for eps); the final sample streams in
    single-k-block chunks to shorten the end-of-kernel chain.
"""

import numpy as np
from contextlib import ExitStack

import concourse.mybir as mybir
import concourse.tile as tile
from concourse import bacc
from concourse.bass_utils import run_bass_kernel_spmd

B, IN, OUT = 128, 1024, 1024
N_CORES = 8
BP = B // N_CORES          # 16 batch rows per core
P = 128                    # partitions
KB = IN // P               # 8 k-blocks
NHALF = 512                # PSUM-bank-sized matmul moving free dim
CHUNK_K = 4                # k-blocks per eps chunk (steady state)

f32 = mybir.dt.float32
f16 = mybir.dt.float16
MULT = mybir.AluOpType.mult
ADD = mybir.AluOpType.add
ACT = mybir.ActivationFunctionType

EPS_BUFS = 8               # eps stream tile slots
ER_BUFS = 3                # eps*sigma product slots
REP = 1                    # body repetitions (>1 only for timing experiments)

_compiled = {}


def build(rep=None):
    rep = REP if rep is None else rep
    nc = bacc.Bacc("TRN2", debug=False, enable_asserts=False)

    eps_d = nc.dram_tensor("eps", (BP, IN, OUT), f16, kind="ExternalInput").ap()
    xTp_d = nc.dram_tensor("xTp", (P, KB, BP), f16, kind="ExternalInput").ap()
    mu_d = nc.dram_tensor("mu", (KB, P, OUT), f16, kind="ExternalInput").ap()
    ro_d = nc.dram_tensor("ro", (KB, P, OUT), f16, kind="ExternalInput").ap()
    eb_d = nc.dram_tensor("ebias", (BP, OUT), f16, kind="ExternalInput").ap()
    rb_d = nc.dram_tensor("robias", (BP, OUT), f16, kind="ExternalInput").ap()
    mb_d = nc.dram_tensor("mubias", (BP, OUT), f16, kind="ExternalInput").ap()
    id_d = nc.dram_tensor("ident", (BP, BP), f16, kind="ExternalInput").ap()
    out_d = nc.dram_tensor("out", (BP, OUT), f32, kind="ExternalOutput").ap()

    # eps as [b][p, k, o] (i = k*128 + p on partitions)
    eps_r = eps_d.rearrange("b (k p) o -> b p k o", p=P)
    ro_r = ro_d.rearrange("k p o -> p k o")
    mu_r = mu_d.rearrange("k p o -> p k o")

    with tile.TileContext(nc) as tc, ExitStack() as ctx:
        consts = ctx.enter_context(tc.tile_pool(name="consts", bufs=1))
        small = ctx.enter_context(tc.tile_pool(name="small", bufs=1))
        eps_pool = ctx.enter_context(tc.tile_pool(name="eps_pool", bufs=1))
        psum_pool = ctx.enter_context(tc.tile_pool(name="psum", bufs=1, space="PSUM"))

        for _rep in range(rep):
            # ---- bulk weights first on the sync ring, ahead of eps ----
            ro_all = consts.tile([P, KB, OUT], f16)
            nc.sync.dma_start(ro_all[:], ro_r)
            mu_all = consts.tile([P, KB, OUT], f16)
            nc.sync.dma_start(mu_all[:], mu_r)

            # tiny constants ride SWDGE
            xTp = consts.tile([P, KB, BP], f16)
            nc.gpsimd.dma_start(xTp[:], xTp_d)
            ident = consts.tile([BP, BP], f16)
            nc.gpsimd.dma_start(ident[:], id_d)
            eb16 = small.tile([BP, OUT], f16)
            nc.gpsimd.dma_start(eb16[:], eb_d)
            rb16 = small.tile([BP, OUT], f16)
            nc.gpsimd.dma_start(rb16[:], rb_d)
            mb16 = small.tile([BP, OUT], f16)
            nc.gpsimd.dma_start(mb16[:], mb_d)

            # sigma = softplus(ro) = ln(1 + exp(ro)). All Exp ops are
            # batched before all Ln ops so the act-table pass inserts
            # exactly two LoadActFuncSet (Exp table, then Ln table);
            # everything else on ScalarE is Copy, present in every table.
            exp_all = consts.tile([P, KB, OUT], f32)
            for k in range(KB):
                nc.scalar.activation(exp_all[:, k, :], ro_all[:, k, :], ACT.Exp)
            exp_b = small.tile([BP, OUT], f32)
            nc.scalar.activation(exp_b[:], rb16[:], ACT.Exp)
            sigma_all = consts.tile([P, KB, OUT], f16)
            for k in range(KB):
                nc.scalar.activation(
                    sigma_all[:, k, :], exp_all[:, k, :], ACT.Ln, bias=1.0
                )
            sb16 = small.tile([BP, OUT], f16)
            nc.scalar.activation(sb16[:], exp_b[:], ACT.Ln, bias=1.0)

            # mu term: psum_mu[m, o] = sum_i x[m, i] * mu[i, o]
            psum_mu = psum_pool.tile([BP, OUT], f32, tag="pmu", bufs=1, name="psum_mu")
            for k in range(KB):
                for h in range(2):
                    nc.tensor.matmul(
                        psum_mu[:, h * NHALF : (h + 1) * NHALF],
                        xTp[:, k, :],
                        mu_all[:, k, h * NHALF : (h + 1) * NHALF],
                        start=(k == 0),
                        stop=(k == KB - 1),
                    )
            mu_s = small.tile([BP, OUT], f16)
            nc.scalar.activation(mu_s[:], psum_mu[:], ACT.Copy)

            # bias rows: base16 = ebias * softplus(robias) + mubias + x@mu.
            # Emitted after row 0's eps products so these DVE ops don't
            # head-of-line block the eps stream on the DVE queue.
            base16 = small.tile([BP, OUT], f16)

            def emit_base16():
                nc.vector.tensor_tensor(base16[:], eb16[:], sb16[:], MULT)
                nc.vector.tensor_tensor(base16[:], base16[:], mb16[:], ADD)
                nc.vector.tensor_tensor(base16[:], base16[:], mu_s[:], ADD)

            # ---- main loop: one PSUM row per batch sample ----
            for b in range(BP):
                ck = 1 if b == BP - 1 else CHUNK_K
                prow = psum_pool.tile([1, OUT], f32, tag="prow", bufs=3, name="prow")
                for c in range(KB // ck):
                    ksl = slice(c * ck, (c + 1) * ck)
                    et = eps_pool.tile(
                        [P, ck, OUT], f16, tag="eps_t", name="et", bufs=EPS_BUFS
                    )
                    nc.sync.dma_start(et[:], eps_r[b][:, ksl, :])
                    er = eps_pool.tile(
                        [P, ck, OUT], f16, tag="eps_r", name="er", bufs=ER_BUFS
                    )
                    nc.vector.tensor_tensor(er[:], et[:], sigma_all[:, ksl, :], MULT)
                    for kk in range(ck):
                        k = c * ck + kk
                        for h in range(2):
                            nc.tensor.matmul(
                                prow[:, h * NHALF : (h + 1) * NHALF],
                                xTp[:, k, b : b + 1],
                                er[:, kk, h * NHALF : (h + 1) * NHALF],
                                start=(k == 0),
                                stop=False,
                            )
                if b == 0:
                    emit_base16()
                # fold bias row b into the PSUM row: ident[:, b] selects it
                for h in range(2):
                    nc.tensor.matmul(
                        prow[:, h * NHALF : (h + 1) * NHALF],
                        ident[:, b : b + 1],
                        base16[:, h * NHALF : (h + 1) * NHALF],
                        start=False,
                        stop=True,
                    )
                orow = eps_pool.tile([1, OUT], f32, tag="orow", bufs=3, name="orow")
                nc.scalar.activation(orow[:], prow[:], ACT.Copy)
                nc.scalar.dma_start(out_d[b : b + 1, :], orow[:])

    nc.compile()
    return nc


def get_nc(rep=None):
    rep = REP if rep is None else rep
    key = (CHUNK_K, EPS_BUFS, ER_BUFS, rep)
    if key not in _compiled:
        _compiled[key] = build(rep)
    return _compiled[key]


def make_in_maps(x, eps, eps_bias, mu, ro, mu_bias, ro_bias):
    x = np.asarray(x, dtype=np.float32)
    eps = np.asarray(eps)
    eps_bias = np.asarray(eps_bias, dtype=np.float32)
    mu16 = np.ascontiguousarray(
        np.asarray(mu, dtype=np.float16).reshape(KB, P, OUT)
    )
    ro16 = np.ascontiguousarray(
        np.asarray(ro, dtype=np.float16).reshape(KB, P, OUT)
    )
    mu_b = np.ascontiguousarray(
        np.broadcast_to(
            np.asarray(mu_bias, dtype=np.float16).reshape(1, OUT), (BP, OUT)
        )
    )
    ro_b = np.ascontiguousarray(
        np.broadcast_to(
            np.asarray(ro_bias, dtype=np.float16).reshape(1, OUT), (BP, OUT)
        )
    )
    ident = np.eye(BP, dtype=np.float16)
    in_maps = []
    for c in range(N_CORES):
        sl = slice(c * BP, (c + 1) * BP)
        # x rows for this core as [p, k, m]: x[sl].T is (IN, BP) = (k*P, m)
        xTp = np.ascontiguousarray(
            x[sl].T.astype(np.float16).reshape(KB, P, BP).transpose(1, 0, 2)
        )
        in_maps.append(
            {
                "eps": np.ascontiguousarray(eps[sl], dtype=np.float16),
                "xTp": xTp,
                "mu": mu16,
                "ro": ro16,
                "ebias": np.ascontiguousarray(eps_bias[sl], dtype=np.float16),
                "robias": ro_b,
                "mubias": mu_b,
                "ident": ident,
            }
        )
    return in_maps


def run(trace=False, **inputs):
    nc = get_nc()
    in_maps = make_in_maps(**inputs)
    res = run_bass_kernel_spmd(
        nc, in_maps, core_ids=list(range(N_CORES)), trace=trace
    )
    out = np.concatenate([r["out"] for r in res.results], axis=0)
    return out, res


def kernel(**inputs) -> np.ndarray:
    out, _ = run(trace=False, **inputs)
    return out


# revision 8
# speedup vs baseline: 1.8609x; 1.1260x over previous
"""Trainium2 Bass kernel for nn_BayesianLayer (Bayesian linear layer).

Math (per batch row b):
    sigma      = softplus(ro)                          # (IN, OUT)
    weights_b  = eps_b * sigma + mu                    # (IN, OUT)
    bias_b     = eps_bias_b * softplus(ro_bias) + mu_bias
    out_b      = x_b @ weights_b + bias_b              # (OUT,)

Sharding: data-parallel over the batch dim across 8 NeuronCores
(16 rows each); mu/ro/biases replicated.

The kernel is DMA-bound on streaming eps (the cost model serializes
all DMA at ~360 B/ns), so eps/mu/ro/x/biases are staged host-side in
fp16 (the rel-err budget is 2e-2; fp16 staging costs ~5e-4).
Per-core HBM traffic drops from ~72.8 MB to ~38 MB.

Per-core device kernel:
  - DMA order on the sync ring: two packed small tensors (x columns +
    identity; the three bias rows), then ro, then mu as single 2 MB
    transfers, then the eps stream — so every small input lands in
    the first ~2.5 us and no compute queue head-of-line blocks on a
    transfer stuck behind the stream.
  - sigma = softplus(ro) = ln(1 + exp(ro)) on ScalarE. All Exp ops
    are batched before all Ln ops so the act-table pass inserts
    exactly two LoadActFuncSet; everything else on ScalarE is Copy,
    present in every table.
  - eps streams in [128, 4*1024] fp16 tiles (i on partitions, 4
    k-blocks per tile, 8 rotating slots so slot-free semaphores run
    ahead of the DMA ring). VectorE computes eps * sigma with a fp16
    tensor_tensor (2x DVE fast mode). TensorE contracts each k-block
    with the sample's x column as the (free-to-load) stationary,
    accumulating into a [1, 1024] PSUM row.
  - the mu term (x @ mu) is one M=16 fp16 matmul phase; its PSUM
    result folds into the bias rows (bias = eps_bias *
    softplus(ro_bias) + mu_bias + x@mu), which are added into each
    sample's PSUM row by a 16-partition matmul against an identity
    column right after the k=0 matmuls (off the end-of-row chain).
  - PSUM rows leave via a ScalarE copy + DMA on the scalar ring; the
    final sample streams in single-k-block chunks to shorten the
    end-of-kernel chain.
"""

import numpy as np
from contextlib import ExitStack

import concourse.mybir as mybir
import concourse.tile as tile
from concourse import bacc
from concourse.bass_utils import run_bass_kernel_spmd

B, IN, OUT = 128, 1024, 1024
N_CORES = 8
BP = B // N_CORES          # 16 batch rows per core
P = 128                    # partitions
KB = IN // P               # 8 k-blocks
NHALF = 512                # PSUM-bank-sized matmul moving free dim
CHUNK_K = 4                # k-blocks per eps chunk (steady state)
XW = KB * BP               # x columns in the packed small tensor
BIGW = XW + BP             # + identity columns

f32 = mybir.dt.float32
f16 = mybir.dt.float16
MULT = mybir.AluOpType.mult
ADD = mybir.AluOpType.add
ACT = mybir.ActivationFunctionType

EPS_BUFS = 8               # eps stream tile slots
ER_BUFS = 3                # eps*sigma product slots
REP = 1                    # body repetitions (>1 only for timing experiments)

_compiled = {}


def build(rep=None):
    rep = REP if rep is None else rep
    nc = bacc.Bacc("TRN2", debug=False, enable_asserts=False)

    eps_d = nc.dram_tensor("eps", (BP, IN, OUT), f16, kind="ExternalInput").ap()
    big_d = nc.dram_tensor("big", (P, BIGW), f16, kind="ExternalInput").ap()
    bias_d = nc.dram_tensor("bias3", (BP, 3, OUT), f16, kind="ExternalInput").ap()
    mu_d = nc.dram_tensor("mu", (KB, P, OUT), f16, kind="ExternalInput").ap()
    ro_d = nc.dram_tensor("ro", (KB, P, OUT), f16, kind="ExternalInput").ap()
    out_d = nc.dram_tensor("out", (BP, OUT), f32, kind="ExternalOutput").ap()

    # eps as [b][p, k, o] (i = k*128 + p on partitions)
    eps_r = eps_d.rearrange("b (k p) o -> b p k o", p=P)
    ro_r = ro_d.rearrange("k p o -> p k o")
    mu_r = mu_d.rearrange("k p o -> p k o")

    with tile.TileContext(nc) as tc, ExitStack() as ctx:
        consts = ctx.enter_context(tc.tile_pool(name="consts", bufs=1))
        small = ctx.enter_context(tc.tile_pool(name="small", bufs=1))
        eps_pool = ctx.enter_context(tc.tile_pool(name="eps_pool", bufs=1))
        psum_pool = ctx.enter_context(tc.tile_pool(name="psum", bufs=1, space="PSUM"))

        for _rep in range(rep):
            # ---- small tensors first on the sync ring ----
            # big: x columns [p, k*16 + m] then identity columns
            big = consts.tile([P, BIGW], f16)
            nc.sync.dma_start(big[:], big_d)
            bias3 = small.tile([BP, 3, OUT], f16)
            nc.sync.dma_start(bias3[:], bias_d)

            def xcol(k, b):
                return big[:, k * BP + b : k * BP + b + 1]

            # ---- bulk weights, still ahead of the eps stream ----
            ro_all = consts.tile([P, KB, OUT], f16)
            nc.sync.dma_start(ro_all[:], ro_r)
            mu_all = consts.tile([P, KB, OUT], f16)
            nc.sync.dma_start(mu_all[:], mu_r)

            # sigma = softplus(ro) = ln(1 + exp(ro)); Exp batch then Ln batch
            exp_all = consts.tile([P, KB, OUT], f32)
            for k in range(KB):
                nc.scalar.activation(exp_all[:, k, :], ro_all[:, k, :], ACT.Exp)
            exp_b = small.tile([BP, OUT], f32)
            nc.scalar.activation(exp_b[:], bias3[:, 1, :], ACT.Exp)
            sigma_all = consts.tile([P, KB, OUT], f16)
            for k in range(KB):
                nc.scalar.activation(
                    sigma_all[:, k, :], exp_all[:, k, :], ACT.Ln, bias=1.0
                )
            sb16 = small.tile([BP, OUT], f16)
            nc.scalar.activation(sb16[:], exp_b[:], ACT.Ln, bias=1.0)

            # mu term: psum_mu[m, o] = sum_i x[m, i] * mu[i, o]
            psum_mu = psum_pool.tile([BP, OUT], f32, tag="pmu", bufs=1, name="psum_mu")
            for k in range(KB):
                for h in range(2):
                    nc.tensor.matmul(
                        psum_mu[:, h * NHALF : (h + 1) * NHALF],
                        big[:, k * BP : (k + 1) * BP],
                        mu_all[:, k, h * NHALF : (h + 1) * NHALF],
                        start=(k == 0),
                        stop=(k == KB - 1),
                    )
            mu_s = small.tile([BP, OUT], f16)
            nc.scalar.activation(mu_s[:], psum_mu[:], ACT.Copy)

            # bias rows: base16 = ebias * softplus(robias) + mubias + x@mu.
            # Emitted after row 0's eps products so these DVE ops don't
            # head-of-line block the eps stream on the DVE queue.
            base16 = small.tile([BP, OUT], f16)

            def emit_base16():
                nc.vector.tensor_tensor(base16[:], bias3[:, 0, :], sb16[:], MULT)
                nc.vector.tensor_tensor(base16[:], base16[:], bias3[:, 2, :], ADD)
                nc.vector.tensor_tensor(base16[:], base16[:], mu_s[:], ADD)

            # ---- main loop: one PSUM row per batch sample ----
            for b in range(BP):
                ck = 1 if b == BP - 1 else CHUNK_K
                prow = psum_pool.tile([1, OUT], f32, tag="prow", bufs=3, name="prow")
                for c in range(KB // ck):
                    ksl = slice(c * ck, (c + 1) * ck)
                    et = eps_pool.tile(
                        [P, ck, OUT], f16, tag="eps_t", name="et", bufs=EPS_BUFS
                    )
                    nc.sync.dma_start(et[:], eps_r[b][:, ksl, :])
                    er = eps_pool.tile(
                        [P, ck, OUT], f16, tag="eps_r", name="er", bufs=ER_BUFS
                    )
                    nc.vector.tensor_tensor(er[:], et[:], sigma_all[:, ksl, :], MULT)
                    for kk in range(ck):
                        k = c * ck + kk
                        for h in range(2):
                            nc.tensor.matmul(
                                prow[:, h * NHALF : (h + 1) * NHALF],
                                xcol(k, b),
                                er[:, kk, h * NHALF : (h + 1) * NHALF],
                                start=(k == 0),
                                stop=(k == KB - 1),
                            )
                        if k == 0:
                            # fold bias row b in early, off the tail chain
                            if b == 0:
                                emit_base16()
                            for h in range(2):
                                nc.tensor.matmul(
                                    prow[:, h * NHALF : (h + 1) * NHALF],
                                    big[0:BP, XW + b : XW + b + 1],
                                    base16[:, h * NHALF : (h + 1) * NHALF],
                                    start=False,
                                    stop=False,
                                )
                orow = eps_pool.tile([1, OUT], f32, tag="orow", bufs=3, name="orow")
                nc.scalar.activation(orow[:], prow[:], ACT.Copy)
                nc.scalar.dma_start(out_d[b : b + 1, :], orow[:])

    nc.compile()
    return nc


def get_nc(rep=None):
    rep = REP if rep is None else rep
    key = (CHUNK_K, EPS_BUFS, ER_BUFS, rep)
    if key not in _compiled:
        _compiled[key] = build(rep)
    return _compiled[key]


def make_in_maps(x, eps, eps_bias, mu, ro, mu_bias, ro_bias):
    x = np.asarray(x, dtype=np.float32)
    eps = np.asarray(eps)
    eps_bias = np.asarray(eps_bias, dtype=np.float32)
    mu16 = np.ascontiguousarray(
        np.asarray(mu, dtype=np.float16).reshape(KB, P, OUT)
    )
    ro16 = np.ascontiguousarray(
        np.asarray(ro, dtype=np.float16).reshape(KB, P, OUT)
    )
    mu_b = np.broadcast_to(
        np.asarray(mu_bias, dtype=np.float16).reshape(1, OUT), (BP, OUT)
    )
    ro_b = np.broadcast_to(
        np.asarray(ro_bias, dtype=np.float16).reshape(1, OUT), (BP, OUT)
    )
    in_maps = []
    for c in range(N_CORES):
        sl = slice(c * BP, (c + 1) * BP)
        # x rows for this core as [p, k*16+m]: x[sl].T is (IN, BP) = (k*P, m)
        xTp = x[sl].T.astype(np.float16).reshape(KB, P, BP).transpose(1, 0, 2)
        big = np.zeros((P, BIGW), dtype=np.float16)
        big[:, :XW] = xTp.reshape(P, XW)
        big[:BP, XW:] = np.eye(BP, dtype=np.float16)
        bias3 = np.ascontiguousarray(
            np.stack(
                [eps_bias[sl].astype(np.float16), ro_b, mu_b], axis=1
            )
        )
        in_maps.append(
            {
                "eps": np.ascontiguousarray(eps[sl], dtype=np.float16),
                "big": big,
                "bias3": bias3,
                "mu": mu16,
                "ro": ro16,
            }
        )
    return in_maps


def run(trace=False, **inputs):
    nc = get_nc()
    in_maps = make_in_maps(**inputs)
    res = run_bass_kernel_spmd(
        nc, in_maps, core_ids=list(range(N_CORES)), trace=trace
    )
    out = np.concatenate([r["out"] for r in res.results], axis=0)
    return out, res


def kernel(**inputs) -> np.ndarray:
    out, _ = run(trace=False, **inputs)
    return out
